# revision 17
# baseline (speedup 1.0000x reference)
"""DigitCaps dynamic-routing kernel for 8 Trainium2 NeuronCores.

Strategy: data-parallel over batch (32 per core), W replicated. u_hat is never
materialized; each routing iteration folds the routing coefficients c_ij into a
bf16 copy of W and computes s via a 72-chunk PE matmul with contraction over
(i, r), packed 4-wide into 128x32 column tiles of the PE array (partial sums
collapsed by a ones-block matmul). The per-iteration agreement statistic A[r,c]
(batch mean of <u_hat, v>) is computed as A = sum_{i,d} W ⊙ (p^T v) with the
p^T v matmuls packed 4-wide into 32x128 row tiles, then all-reduced across the
8 cores. The third iteration's b_ij update is dead code in the reference, so
only 2 all-reduces run.
"""
import numpy as np
from contextlib import ExitStack

import concourse.bass as bass
from concourse import bacc
import concourse.tile as tile
from concourse import mybir
from concourse.bass_utils import run_bass_kernel_spmd
from concourse.masks import make_identity

N_CORES = 8
B_FULL, R, C, D, I = 256, 1152, 10, 16, 8
B = B_FULL // N_CORES          # 32 batch per core
G = R // 128                   # 9 chunks of 128 routes
RI = R * I                     # 9216
CD = C * D                     # 160
CDI = C * D * I                # 1280
NUM_IT = 3

FP32 = mybir.dt.float32
BF16 = mybir.dt.bfloat16
ALU = mybir.AluOpType
AX = mybir.AxisListType
AF = mybir.ActivationFunctionType

# storage order of i within W tiles: position k holds i = 4*(k%2) + k//2,
# i.e. [0,4,1,5,2,6,3,7] -- row-tile j=i%4 drains contiguous pairs (i, i+4)
IPOS = {i: (i % 4) * 2 + i // 4 for i in range(I)}


def _build_body(ctx: ExitStack, tc: "tile.TileContext", p_dram, w_dram, v_dram):
    nc = tc.nc

    consts = ctx.enter_context(tc.tile_pool(name="consts", bufs=1))
    pers = ctx.enter_context(tc.tile_pool(name="pers", bufs=1))
    small = ctx.enter_context(tc.tile_pool(name="small", bufs=2))
    dram = ctx.enter_context(tc.tile_pool(name="dram", bufs=2, space="DRAM"))
    ps_4 = ctx.enter_context(tc.tile_pool(name="ps_4", bufs=2, space="PSUM"))
    ps_s = ctx.enter_context(tc.tile_pool(name="ps_s", bufs=1, space="PSUM"))
    ps_y = ctx.enter_context(tc.tile_pool(name="ps_y", bufs=4, space="PSUM"))
    ps_t = ctx.enter_context(tc.tile_pool(name="ps_t", bufs=1, space="PSUM"))

    # p replicated across the 4 partition quadrants (for 32-row PE tiles)
    pb = pers.tile([128, RI], BF16, tag="pb")
    pb_v = pb[:].rearrange("p (g j i) -> p g j i", g=G, j=128, i=I)
    # identity blocks: bf16 [32,32] at quadrant 0 + replication via DMA
    ident = consts.tile([128, B], BF16, tag="ident")
    make_identity(nc, ident[0:B, :])
    for q in range(1, 4):
        nc.sync.dma_start(ident[q * B:(q + 1) * B, :], ident[0:B, :])
    # ones-block collapse matrix, fp32: S4[k, m] = 1 if k % 32 == m
    s4m = consts.tile([128, B], FP32, tag="s4m")
    make_identity(nc, s4m[0:B, :])
    for q in range(1, 4):
        nc.sync.dma_start(s4m[q * B:(q + 1) * B, :], s4m[0:B, :])

    pT = pers.tile([128, I * G * B], BF16, tag="pT")  # [(r,i) chunk, b] blocks
    pT_v = pT[:].rearrange("p (i g b) -> p g i b", i=I, g=G, b=B)
    # W bf16, free order (k, c, d) with k the IPOS-interleaved i position
    wre = [pers.tile([128, CDI], BF16, tag=f"wre{g}", name=f"wre{g}")
           for g in range(G)]

    PCH = RI // G                                     # 1024 p-elems per chunk

    with ExitStack() as s0:
        # fp32 staging pools: released after stage 0
        pstg = s0.enter_context(tc.tile_pool(name="pstg", bufs=3))
        w32p = s0.enter_context(tc.tile_pool(name="w32p", bufs=G))

        # p pipeline, chunked per g so PE transposes start early
        for g in range(G):
            p32 = pstg.tile([B, PCH], FP32, tag="p32")
            nc.gpsimd.dma_start(p32[:], p_dram[:, g * PCH:(g + 1) * PCH])
            dst = pb[0:B, g * PCH:(g + 1) * PCH]
            if g % 2 == 0:
                nc.vector.tensor_copy(dst, p32[:])
            else:
                nc.scalar.copy(dst, p32[:])
            # replicate chunk into quadrants 1..3
            for q in range(1, 4):
                nc.sync.dma_start(pb[q * B:(q + 1) * B, g * PCH:(g + 1) * PCH],
                                  pb[0:B, g * PCH:(g + 1) * PCH])
            tp = ps_t.tile([128, I * B], BF16, tag="tp")
            for i in range(I):
                nc.tensor.transpose(tp[:, i * B:(i + 1) * B],
                                    pb_v[0:B, g, :, i], ident[0:B, :])
            tp_v = tp[:].rearrange("p (i b) -> p i b", i=I, b=B)
            nc.scalar.copy(pT_v[:, g], tp_v)

        # W pipeline: fp32 (c,d,i) -> bf16 (k,c,d), k = IPOS-interleave of i
        for g in range(G):
            w32 = w32p.tile([128, CDI], FP32)
            nc.sync.dma_start(w32[:], w_dram[128 * g:128 * (g + 1), :])
            # src i = 4h + j maps to dst position k = 2j + h
            src_v = w32[:].rearrange("p (c d h j) -> p j h c d",
                                     c=C, d=D, h=2, j=4)
            dst_v = wre[g][:].rearrange("p (j h c d) -> p j h c d",
                                        j=4, h=2, c=C, d=D)
            if g % 2 == 0:
                nc.vector.tensor_copy(dst_v, src_v)
            else:
                nc.scalar.copy(dst_v, src_v)

    wcp = ctx.enter_context(tc.tile_pool(name="wcp", bufs=1))
    work = ctx.enter_context(tc.tile_pool(name="work", bufs=2))

    # routing logits, [128, (g c)] layout
    bij = pers.tile([128, G * C], FP32, tag="bij")
    nc.gpsimd.memset(bij[:], 0.0)
    bij_v = bij[:].rearrange("p (g c) -> p g c", g=G, c=C)

    def w_slice(t, i):
        """[128, (c,d)] contiguous view of a W tile for true i index."""
        k = IPOS[i]
        return t[:, k * CD:(k + 1) * CD]

    # ---------------- routing iterations ----------------
    for t in range(NUM_IT):
        last = t == NUM_IT - 1
        if t == 0:
            wc = wre                      # c_ij uniform: fold 0.1 into squash
            sqrt_e = 0.1
            e_scale = 0.01
        else:
            sqrt_e = 1.0
            e_scale = 1.0
            # softmax over c of bij -> cbb (bf16)
            mx = small.tile([128, G], FP32, tag="mx")
            nc.vector.tensor_reduce(mx[:], bij_v, axis=AX.X, op=ALU.max)
            eb = small.tile([128, G * C], FP32, tag="eb")
            eb_v = eb[:].rearrange("p (g c) -> p g c", g=G, c=C)
            mxb = mx[:].unsqueeze(2).broadcast_to([128, G, C])
            nc.vector.tensor_tensor(eb_v, bij_v, mxb, op=ALU.subtract)
            nc.scalar.activation(eb[:], eb[:], AF.Exp)
            sm = small.tile([128, G], FP32, tag="sm")
            nc.vector.tensor_reduce(sm[:], eb_v, axis=AX.X, op=ALU.add)
            rc = small.tile([128, G], FP32, tag="rc")
            nc.vector.reciprocal(rc[:], sm[:])
            cbb = small.tile([128, G * C], BF16, tag="cbb")
            cbb_v = cbb[:].rearrange("p (g c) -> p g c", g=G, c=C)
            rcb = rc[:].unsqueeze(2).broadcast_to([128, G, C])
            nc.vector.tensor_tensor(cbb_v, eb_v, rcb, op=ALU.mult)

            # Wc[g] = wre[g] * c  (broadcast over i-position and d), on DVE
            wc = [wcp.tile([128, CDI], BF16, tag=f"wc{g}", name=f"wc{g}_{t}")
                  for g in range(G)]
            for g in range(G):
                w4 = wre[g][:].rearrange("p (k c d) -> p k c d", k=I, c=C, d=D)
                o4 = wc[g][:].rearrange("p (k c d) -> p k c d", k=I, c=C, d=D)
                cb4 = cbb[:, g * C:(g + 1) * C].unsqueeze(1).unsqueeze(3) \
                    .broadcast_to([128, I, C, D])
                nc.vector.tensor_tensor(o4, w4, cb4, op=ALU.mult)

        # s partials: 72 chunks packed 4-wide into 128x32 column tiles,
        # accumulated over 18 rounds into one [128, CD] PSUM tile
        sp4 = ps_4.tile([128, CD], FP32, tag="sp4", name=f"sp4_{t}")
        chunks = [(g, i) for g in range(G) for i in range(I)]
        n_rounds = len(chunks) // 4
        for r in range(n_rounds):
            for j in range(4):
                g, i = chunks[4 * r + j]
                k = i * G + g
                nc.tensor.matmul(
                    sp4[32 * j:32 * (j + 1), :],
                    pT[:, k * B:(k + 1) * B],
                    w_slice(wc[g], i),
                    start=(r == 0),
                    stop=(r == n_rounds - 1),
                    tile_position=(0, 32 * j),
                )
        # collapse the 4 quadrant partials: s = S4^T @ sp4
        s4sb = small.tile([128, CD], FP32, tag="s4sb")
        nc.scalar.copy(s4sb[:], sp4[:])
        s_ps = ps_s.tile([B, CD], FP32, tag="s_ps")
        nc.tensor.matmul(s_ps[:], s4m[:], s4sb[:], start=True, stop=True)

        # squash: v = s_eff * sqrt(sq)/(1+sq), sq = |s_eff|^2, s_eff = sqrt_e*s
        s_sb = small.tile([B, CD], FP32, tag="s_sb")
        nc.scalar.copy(s_sb[:], s_ps[:])
        s2 = small.tile([B, CD], FP32, tag="s2")
        nc.vector.tensor_tensor(s2[:], s_sb[:], s_sb[:], op=ALU.mult)
        sq = small.tile([B, C], FP32, tag="sq")
        nc.vector.tensor_reduce(sq[:],
                                s2[:].rearrange("b (c d) -> b c d", c=C, d=D),
                                axis=AX.X, op=ALU.add)
        r1 = small.tile([B, C], FP32, tag="r1")
        nc.scalar.activation(r1[:], sq[:], AF.Sqrt, scale=e_scale)
        den = small.tile([B, C], FP32, tag="den")
        nc.vector.tensor_scalar(den[:], sq[:], e_scale, 1.0, op0=ALU.mult,
                                op1=ALU.add)
        rec = small.tile([B, C], FP32, tag="rec")
        nc.vector.reciprocal(rec[:], den[:])
        fac = small.tile([B, C], FP32, tag="fac")
        nc.vector.tensor_tensor(fac[:], r1[:], rec[:], op=ALU.mult)

        v32 = small.tile([B, CD], FP32, tag="v32")
        fb = fac[:].unsqueeze(2).broadcast_to([B, C, D])
        nc.vector.scalar_tensor_tensor(
            out=v32[:].rearrange("b (c d) -> b c d", c=C, d=D),
            in0=s_sb[:].rearrange("b (c d) -> b c d", c=C, d=D),
            scalar=sqrt_e, op0=ALU.mult, in1=fb, op1=ALU.mult)

        if last:
            nc.sync.dma_start(v_dram[:, :], v32[:])
            continue

        # ---- agreement stats: A[r, c] = sum_{i,d} W ⊙ (p^T v), AllReduce ----
        vb = pers.tile([128, CD], BF16, tag="vb", name=f"vb_{t}")
        nc.scalar.copy(vb[0:B, :], v32[:])
        for q in range(1, 4):
            nc.sync.dma_start(vb[q * B:(q + 1) * B, :], vb[0:B, :])

        Apart = pers.tile([128, G * C], FP32, tag="Apart")
        for g in range(G):
            y_sb = work.tile([128, CDI], BF16, tag="y_sb",
                             name=f"y_sb{g}_{t}")
            # row-tile j handles i = j and i = j+4 -> adjacent k positions
            for j in range(4):
                y_ps = ps_y.tile([128, 2 * CD], FP32, tag="y_ps")
                for h in range(2):
                    i = 4 * h + j
                    nc.tensor.matmul(y_ps[:, h * CD:(h + 1) * CD],
                                     pb_v[32 * j:32 * (j + 1), g, :, i],
                                     vb[32 * j:32 * (j + 1), :],
                                     start=True, stop=True,
                                     tile_position=(32 * j, 0))
                nc.scalar.copy(
                    y_sb[:, 2 * j * CD:(2 * j + 2) * CD], y_ps[:])
            prod = work.tile([128, CDI], BF16, tag="prod",
                             name=f"prod{g}_{t}")
            nc.vector.tensor_tensor(prod[:], wre[g][:], y_sb[:], op=ALU.mult)
            # A_g = sum over (d, i): contiguous d-reduce, then tiny i-reduce
            pg1 = small.tile([128, I * C], FP32, tag="pg1")
            nc.vector.tensor_reduce(
                pg1[:],
                prod[:].rearrange("p (k c d) -> p k c d", k=I, c=C, d=D),
                axis=AX.X, op=ALU.add)
            nc.vector.tensor_reduce(
                Apart[:, g * C:(g + 1) * C],
                pg1[:].rearrange("p (k c) -> p c k", k=I, c=C),
                axis=AX.X, op=ALU.add)

        cc_in = dram.tile([128, G * C], FP32, tag="cc_in")
        cc_out = dram.tile([128, G * C], FP32, tag="cc_out",
                           addr_space="Shared")
        nc.sync.dma_start(cc_in[:], Apart[:])
        nc.gpsimd.collective_compute(
            "AllReduce", ALU.add,
            replica_groups=[list(range(N_CORES))],
            ins=[cc_in[:].opt()],
            outs=[cc_out[:].opt()],
        )
        acc = small.tile([128, G * C], FP32, tag="acc")
        nc.sync.dma_start(acc[:], cc_out[:])
        nc.vector.scalar_tensor_tensor(
            out=bij[:], in0=acc[:], scalar=1.0 / B_FULL, op0=ALU.mult,
            in1=bij[:], op1=ALU.add)


_CACHED = None


def _build():
    global _CACHED
    if _CACHED is not None:
        return _CACHED
    nc = bacc.Bacc("TRN2", target_bir_lowering=False, debug=False,
                   num_devices=N_CORES)
    p_dram = nc.dram_tensor("p_in", [B, RI], FP32, kind="ExternalInput").ap()
    w_dram = nc.dram_tensor("w_in", [R, CDI], FP32, kind="ExternalInput").ap()
    v_dram = nc.dram_tensor("v_out", [B, CD], FP32, kind="ExternalOutput").ap()
    with tile.TileContext(nc) as tc:
        with ExitStack() as ctx:
            _build_body(ctx, tc, p_dram, w_dram, v_dram)
    nc.finalize()
    _CACHED = nc
    return nc


def kernel(prim_caps: np.ndarray, W: np.ndarray, _trace: bool = False):
    assert prim_caps.shape == (B_FULL, R, I) and W.shape == (1, R, C, D, I)
    nc = _build()
    p_flat = np.ascontiguousarray(prim_caps.reshape(B_FULL, RI).astype(np.float32))
    w_flat = np.ascontiguousarray(W.reshape(R, CDI).astype(np.float32))
    in_maps = [
        {"p_in": np.ascontiguousarray(p_flat[k * B:(k + 1) * B]), "w_in": w_flat}
        for k in range(N_CORES)
    ]
    res = run_bass_kernel_spmd(nc, in_maps, core_ids=list(range(N_CORES)),
                               trace=_trace)
    out = np.concatenate(
        [res.results[k]["v_out"].reshape(B, C, D, 1) for k in range(N_CORES)],
        axis=0)
    if _trace:
        return out, res
    return out
